# revision 14
# baseline (speedup 1.0000x reference)
"""GNN message-passing kernel for 8 Trainium2 NeuronCores.

Strategy: shard nodes/edges by destination-node range across 8 cores;
replicate weights; per layer all-gather the W_msg-transformed node table
(bf16) and gather per-edge rows via dma_gather; scatter-add via one-hot
matmuls on the tensor engine with PSUM accumulation per 128-node window.
One-hot scatter matrices are precomputed on the host and streamed from
DRAM (the DVE per-partition-scalar EQ build is ~1.6us/chunk on HW).
Edge-attr aggregate (computed once in layer 0), degree-bias and self
terms are folded into the same PSUM accumulation; ReLU evicts PSUM via
the scalar engine.
"""
import sys, os
for p in ('/opt/trn_rl_repo', '/root/.axon_site', '/root/.axon_site/_ro/trn_rl_repo',
          '/root/.axon_site/_ro/pypackages'):
    if os.path.isdir(p) and p not in sys.path:
        sys.path.append(p)

import numpy as np

# ---------------- problem constants (hardcoded) ----------------
N, E, G = 100000, 1000000, 64
F_IN, H, E_IN, GF = 84, 128, 6, 10368
C = 8                 # cores
SH = N // C           # 12500 real nodes per core
T = 98                # node chunks per core (ceil(12500/128))
SHP = T * 128         # 12544 padded nodes per core
NP = C * SHP          # padded global node space (100352)
NB = 4                # src buckets of 25088 rows (2 shards each, int16-safe)
BROWS = NP // NB      # 25088
W = T                 # dst windows per core (128 nodes each)
CALL = 2048           # slots per dma_gather call


def _host_prep(inputs):
    x = np.asarray(inputs['x'], np.float32)
    ei = np.asarray(inputs['edge_index']).astype(np.int64)
    ea = np.asarray(inputs['edge_attr'], np.float32)
    batch = np.asarray(inputs['batch']).astype(np.int64)

    src, dst = ei[0], ei[1]
    # table row: node (cs, local r = t*128+p) lives at quarter q = p//32:
    #   row = q*BROWS + cs*3136 + (p%32)*T + t   (matches per-quarter AllGather)
    cs = src // SH
    r = src - cs * SH
    p_, t_ = r % 128, r // 128
    q_ = p_ // 32
    bucket = q_
    idx_in_bucket = (cs * 3136 + (p_ % 32) * T + t_).astype(np.int16)

    core = dst // SH
    dst_local = dst - core * SH
    win = dst_local >> 7
    rel = dst_local & 127

    # per-(core,bucket,window) counts -> chunk plan (same on all cores)
    flat = (core * NB + bucket) * W + win
    cnt = np.bincount(flat, minlength=C * NB * W).reshape(C, NB, W)
    cell_chunks = -(-cnt.max(axis=0) // 128)          # [NB, W] (can be 0)
    cell_chunks = np.maximum(cell_chunks, 1)
    # bucket-major chunk layout for gather slots: (b, w) cells in w order
    cell_base = np.zeros((NB, W), np.int64)           # chunk base within bucket
    for b in range(NB):
        cell_base[b] = np.concatenate([[0], np.cumsum(cell_chunks[b])[:-1]])
    bucket_chunks = cell_chunks.sum(axis=1)           # [NB]
    bucket_base = np.concatenate([[0], np.cumsum(bucket_chunks)[:-1]])
    KTOT = int(bucket_chunks.sum())
    TOTS = KTOT * 128
    # window-major order for st/ea streams: chunks ordered (w, b, i)
    # global chunk id (bucket-major) -> position in window-major stream
    wm_of_bm = np.zeros(KTOT, np.int64)
    pos = 0
    win_nch = np.zeros(W, np.int64)
    for w in range(W):
        for b in range(NB):
            nch = int(cell_chunks[b, w])
            base = int(bucket_base[b] + cell_base[b, w])
            for i in range(nch):
                wm_of_bm[base + i] = pos
                pos += 1
            win_nch[w] += nch
    win_base = np.concatenate([[0], np.cumsum(win_nch)[:-1]])

    meta = dict(cell_chunks=cell_chunks, cell_base=cell_base,
                bucket_chunks=bucket_chunks, bucket_base=bucket_base,
                KTOT=KTOT, TOTS=TOTS, win_nch=win_nch, win_base=win_base)

    deg_real = np.bincount(dst, minlength=N)

    per_core = []
    order_all = np.lexsort((dst_local, bucket, core))
    core_sorted = core[order_all]
    core_starts = np.searchsorted(core_sorted, np.arange(C + 1))
    for c in range(C):
        sel = order_all[core_starts[c]:core_starts[c + 1]]
        b_c, w_c, rel_c = bucket[sel], win[sel], rel[sel]
        # rank within (b,w) cell (sel already sorted by (b, w, dst_local))
        cellid = b_c * W + w_c
        uniq, first_pos, counts = np.unique(cellid, return_index=True,
                                            return_counts=True)
        rank = np.arange(len(sel)) - np.repeat(first_pos, counts)
        slot = (bucket_base[b_c] + cell_base[b_c, w_c]) * 128 + rank
        chunk_bm = slot >> 7
        chunk_wm = wm_of_bm[chunk_bm]
        p_in_chunk = slot & 127

        idx_flat = np.zeros(TOTS, np.int16)
        idx_flat[slot] = idx_in_bucket[sel]
        idx16 = np.tile(idx_flat.reshape(-1, 16).T, (8, 1)).copy()  # [128,TOTS/16]

        # one-hot st: [128 p, KTOT(wm), 128 rel] bf16
        import ml_dtypes
        st = np.zeros((128, KTOT, 128), ml_dtypes.bfloat16)
        st[p_in_chunk, chunk_wm, rel_c] = 1.0
        st_flat = st.reshape(128, KTOT * 128)

        ea_arr = np.zeros((128, KTOT, 6), ml_dtypes.bfloat16)
        ea_arr[p_in_chunk, chunk_wm, :] = ea[sel].astype(ml_dtypes.bfloat16)
        ea_flat = ea_arr.reshape(128, KTOT * 6)

        # deg row [1, SHP], batch one-hot [128, T*G]
        degv = np.zeros((1, SHP), ml_dtypes.bfloat16)
        degv[0, :SH] = deg_real[c * SH:(c + 1) * SH].astype(ml_dtypes.bfloat16)
        ph = np.zeros((128, T, G), ml_dtypes.bfloat16)
        nn = np.arange(SH)
        ph[nn % 128, nn // 128, batch[c * SH:(c + 1) * SH]] = 1.0
        ph_flat = ph.reshape(128, T * G)

        xt = np.zeros((F_IN + 1, SHP), ml_dtypes.bfloat16)
        xt[:F_IN, :SH] = x[c * SH:(c + 1) * SH].T.astype(ml_dtypes.bfloat16)
        xt[F_IN, :] = 1.0
        per_core.append(dict(idx16=idx16, st=st_flat, ea=ea_flat,
                             deg=degv, ph=ph_flat, xt=xt))

    import ml_dtypes
    bf = lambda a: np.asarray(a, np.float32).astype(ml_dtypes.bfloat16)
    cntg = np.bincount(batch, minlength=G).astype(np.float32)
    shared = dict(invcnt=(1.0 / np.maximum(cntg, 1.0)).reshape(G, 1))

    def wedge_ext(we, bsum):
        return np.concatenate([np.asarray(we, np.float32),
                               np.asarray(bsum, np.float32).reshape(1, -1)], 0)
    shared['wmsg1'] = bf(inputs['W_msg1'])
    shared['wmsg2'] = bf(inputs['W_msg2'])
    shared['wmsg3'] = bf(inputs['W_msg3'])
    shared['wedge1'] = bf(wedge_ext(inputs['W_edge1'],
                                    np.asarray(inputs['b_edge1']) +
                                    np.asarray(inputs['b_msg1'])))
    shared['wedge2'] = bf(wedge_ext(inputs['W_edge2'],
                                    np.asarray(inputs['b_edge2']) +
                                    np.asarray(inputs['b_msg2'])))
    shared['wedge3'] = bf(wedge_ext(inputs['W_edge3'],
                                    np.asarray(inputs['b_edge3']) +
                                    np.asarray(inputs['b_msg3'])))
    shared['wself1'] = bf(np.concatenate(
        [np.asarray(inputs['W_self1'], np.float32),
         np.asarray(inputs['b_self1'], np.float32).reshape(1, -1)], 0))
    shared['gaT'] = bf(np.ascontiguousarray(
        np.asarray(inputs['graph_attr'], np.float32).T))          # [GF, 64]
    shared['wg'] = bf(inputs['W_g'])                               # [GF, H]
    shared['bg'] = bf(np.asarray(inputs['b_g']).reshape(1, H))
    shared['wc'] = bf(inputs['W_c'])                               # [256, H]
    shared['bc'] = bf(np.asarray(inputs['b_c']).reshape(1, H))
    shared['wl'] = bf(inputs['W_l'])                               # [H, 1]
    shared['bl'] = bf(np.asarray(inputs['b_l']).reshape(1, 1))
    return meta, per_core, shared


def _build(meta):
    import concourse.bacc as bacc
    import concourse.mybir as mybir
    import concourse.tile as tile
    from concourse.masks import make_identity
    F32, BF16, I16 = mybir.dt.float32, mybir.dt.bfloat16, mybir.dt.int16
    ADD = mybir.AluOpType.add
    MULT = mybir.AluOpType.mult
    RELU = mybir.ActivationFunctionType.Relu

    cell_chunks = meta['cell_chunks']
    cell_base = meta['cell_base']
    bucket_chunks = meta['bucket_chunks']
    bucket_base = meta['bucket_base']
    KTOT, TOTS = meta['KTOT'], meta['TOTS']
    win_nch, win_base = meta['win_nch'], meta['win_base']

    nc = bacc.Bacc("TRN2", target_bir_lowering=False, debug=False, num_devices=C)

    def inp(name, shape, dt=BF16):
        return nc.dram_tensor(name, shape, dt, kind="ExternalInput")
    xt_d = inp('xt', [F_IN + 1, SHP])
    idx_d = inp('idx16', [128, TOTS // 16], I16)
    st_d = inp('st', [128, KTOT * 128])
    ea_d = inp('ea', [128, KTOT * 6])
    deg_d = inp('deg', [1, SHP])
    ph_d = inp('ph', [128, T * G])
    invc_d = inp('invcnt', [G, 1], F32)
    wmsg_d = [inp('wmsg1', [F_IN, H]), inp('wmsg2', [H, H]), inp('wmsg3', [H, H])]
    wedge_d = [inp('wedge1', [7, H]), inp('wedge2', [7, H]), inp('wedge3', [7, H])]
    wself_d = inp('wself1', [F_IN + 1, H])
    gaT_d = inp('gaT', [GF, G])
    wg_d = inp('wg', [GF, H])
    bg_d = inp('bg', [1, H])
    wc_d = inp('wc', [2 * H, H])
    bc_d = inp('bc', [1, H])
    wl_d = inp('wl', [H, 1])
    bl_d = inp('bl', [1, 1])
    out_d = nc.dram_tensor('out', [G, 1], F32, kind="ExternalOutput")

    # gather call plan: round-robin across buckets by call index
    GCALLS = []
    for b in range(NB):
        sz = int(bucket_chunks[b]) * 128
        o = 0
        while o < sz:
            n_ = min(CALL, sz - o)
            GCALLS.append((b, o, n_))
            o += n_
    GCALLS.sort(key=lambda t: (t[1] // CALL, t[0]))

    with tile.TileContext(nc) as tc:
        NCHMAX = int(win_nch.max())
        with tc.tile_pool(name="cst", bufs=1) as cst, \
             tc.tile_pool(name="xbuf", bufs=1) as xbuf, \
             tc.tile_pool(name="xmp", bufs=1) as xmp, \
             tc.tile_pool(name="gp", bufs=2) as gp, \
             tc.tile_pool(name="stp", bufs=4) as stp, \
             tc.tile_pool(name="sp", bufs=4) as sp, \
             tc.tile_pool(name="hp", bufs=1) as hp, \
             tc.tile_pool(name="ps", bufs=3, space="PSUM") as ps, \
             tc.tile_pool(name="ps2", bufs=2, space="PSUM") as ps2, \
             tc.tile_pool(name="ps3", bufs=1, space="PSUM") as ps3, \
             tc.tile_pool(name="dram", bufs=1, space="DRAM") as dram:

            # ---- persistent SBUF data ----
            idx_t = cst.tile([128, TOTS // 16], I16)
            nc.sync.dma_start(idx_t[:], idx_d[:])
            ph_t = cst.tile([128, T, G], BF16)
            nc.sync.dma_start(ph_t[:], ph_d[:].rearrange("p (t g) -> p t g", g=G))
            invc_t = cst.tile([G, 1], F32)
            nc.sync.dma_start(invc_t[:], invc_d[:])
            ident = cst.tile([128, 128], BF16)
            make_identity(nc, ident[:])
            wmsg_t = []
            for l in range(3):
                kk = F_IN if l == 0 else H
                wt = cst.tile([kk, H], BF16, tag=f"wmsg{l}")
                nc.sync.dma_start(wt[:], wmsg_d[l][:])
                wmsg_t.append(wt)
            wedge_t = []
            for l in range(3):
                wt = cst.tile([7, H], BF16, tag=f"wedge{l}")
                nc.sync.dma_start(wt[:], wedge_d[l][:])
                wedge_t.append(wt)
            wedgeD0 = cst.tile([1, H], BF16, tag="wedgeD0")
            nc.sync.dma_start(wedgeD0[:], wedge_d[0][6:7, :])
            wself_t = cst.tile([F_IN + 1, H], BF16)
            nc.sync.dma_start(wself_t[:], wself_d[:])

            A = xbuf.tile([128, SHP], BF16, tag="A")
            B = xbuf.tile([128, SHP], BF16, tag="B")
            nc.vector.memset(A[:], 0.0)
            nc.sync.dma_start(A[:F_IN + 1, :], xt_d[:])

            xm_shard = dram.tile([SHP, H], BF16, tag="xmsh")
            xmf = []
            for q in range(NB):
                xmf_q = dram.tile([BROWS, H], BF16, tag=f"xmf{q}", name=f"xmf{q}")
                xmf.append(xmf_q)
            agg7 = dram.tile([7, SHP], BF16, tag="agg7")   # rows 0-5 sum(ea), 6 deg

            def layer(l, XS, XD, resid, pool_ctx=None):
                KX = F_IN if l == 0 else H
                # 1) xm = XS @ Wmsg (bf16) -> DRAM shard -> AllGather
                xm_sb = xmp.tile([128, T, H], BF16, tag="xmsb")
                for t in range(T):
                    pxm = ps2.tile([128, H], F32, tag="aux")
                    nc.tensor.matmul(pxm[:], XS[:KX, t * 128:(t + 1) * 128],
                                     wmsg_t[l][:], start=True, stop=True)
                    nc.scalar.activation(xm_sb[:, t, :], pxm[:],
                                         mybir.ActivationFunctionType.Copy)
                nc.sync.dma_start(
                    xm_shard[:].rearrange("(p t) f -> p t f", p=128), xm_sb[:])
                for q in range(NB):
                    nc.gpsimd.collective_compute(
                        "AllGather", mybir.AluOpType.bypass,
                        replica_groups=[list(range(C))],
                        ins=[xm_shard[q * 3136:(q + 1) * 3136, :].opt()],
                        outs=[xmf[q][:].opt()])

                # 2) per-edge gathers (bucket streams, round-robin calls)
                gt = {}
                for (b, o, n_) in GCALLS:
                    g = gp.tile([128, CALL // 128, H], BF16, tag=f"g{b}")
                    base_slot = int(bucket_base[b]) * 128
                    nc.gpsimd.dma_gather(
                        g[:, :n_ // 128, :],
                        xmf[b][:],
                        idx_t[:, (base_slot + o) // 16:(base_slot + o + n_) // 16],
                        n_, n_, H, single_packet=False)
                    for cch in range(n_ // 128):
                        gt[(b, (o // 128) + cch)] = g[:, cch, :]

                # 3) scatter windows with folded edge/deg/self terms
                for w in range(W):
                    nch = int(win_nch[w])
                    wb = int(win_base[w])
                    # stream st (and ea at l0) for this window's chunks
                    stw = stp.tile([128, NCHMAX, 128], BF16, tag="stw")
                    nc.sync.dma_start(
                        stw[:, :nch, :],
                        st_d[:, wb * 128:(wb + nch) * 128].rearrange(
                            "p (c f) -> p c f", f=128))
                    if l == 0:
                        eaw = stp.tile([128, NCHMAX, 6], BF16, tag="eaw")
                        nc.sync.dma_start(
                            eaw[:, :nch, :],
                            ea_d[:, wb * 6:(wb + nch) * 6].rearrange(
                                "p (c f) -> p c f", f=6))
                        degw = sp.tile([1, 128], BF16, tag="degw")
                        nc.sync.dma_start(degw[:], deg_d[:, w * 128:(w + 1) * 128])
                    else:
                        agw = sp.tile([7, 128], BF16, tag="agw")
                        nc.sync.dma_start(agw[:], agg7[:, w * 128:(w + 1) * 128])

                    px = ps.tile([128, 128], F32, tag="px")
                    if l == 0:
                        pe = ps2.tile([6, 128], F32, tag="aux")
                    k = 0
                    for b in range(NB):
                        cb = int(cell_base[b, w])
                        for i in range(int(cell_chunks[b, w])):
                            msg = gt[(b, cb + i)]
                            stc = stw[:, k, :]  # window-major k-th chunk
                            nc.tensor.matmul(px[:], msg, stc,
                                             start=(k == 0), stop=False)
                            if l == 0:
                                nc.tensor.matmul(pe[:], eaw[:, k, :], stc,
                                                 start=(k == 0), stop=(k == nch - 1))
                            k += 1
                    # fold terms
                    if l == 0:
                        pesb = sp.tile([6, 128], BF16, tag="pesb")
                        nc.vector.tensor_copy(pesb[:], pe[:])
                        nc.sync.dma_start(agg7[0:6, w * 128:(w + 1) * 128], pesb[:])
                        nc.sync.dma_start(agg7[6:7, w * 128:(w + 1) * 128], degw[:])
                        nc.tensor.matmul(px[:], wedge_t[0][0:6, :], pesb[:],
                                         start=False, stop=False)
                        nc.tensor.matmul(px[:], wedgeD0[:], degw[:],
                                         start=False, stop=False)
                        nc.tensor.matmul(px[:], wself_t[:],
                                         A[:F_IN + 1, w * 128:(w + 1) * 128],
                                         start=False, stop=True)
                    else:
                        nc.tensor.matmul(px[:], wedge_t[l][:], agw[:],
                                         start=False, stop=False)
                        nc.tensor.matmul(px[:], ident[:],
                                         XS[:, w * 128:(w + 1) * 128],
                                         start=False, stop=True)
                    wc_ = slice(w * 128, (w + 1) * 128)
                    nc.scalar.activation(XD[:, wc_], px[:], RELU)
                    if resid:
                        nc.vector.tensor_tensor(out=XD[:, wc_], in0=XD[:, wc_],
                                                in1=XS[:, wc_], op=ADD)
                    if pool_ctx is not None:
                        ppool_, ph_ = pool_ctx
                        ptr = ps2.tile([128, 128], BF16, tag="aux")
                        nc.tensor.transpose(ptr[:], XD[:, wc_], ident[:])
                        xts = sp.tile([128, 128], BF16, tag="xts")
                        nc.scalar.activation(xts[:], ptr[:],
                                             mybir.ActivationFunctionType.Copy)
                        nc.tensor.matmul(ppool_[:], ph_[:, w, :], xts[:],
                                         start=(w == 0), stop=(w == W - 1))

            layer(0, A, B, False)   # x1 in B
            layer(1, B, A, True)    # x2 in A
            ppool = ps3.tile([G, H], F32, tag="ppool")
            layer(2, A, B, True, pool_ctx=(ppool, ph_t))   # x3 in B, pooled

            # -------- pooling reduce + head --------
            pool_sb = hp.tile([G, H], F32, tag="poolsb")
            nc.vector.tensor_copy(pool_sb[:], ppool[:])
            cc_in = dram.tile([G, H], F32, tag="ccin")
            cc_out = dram.tile([G, H], F32, tag="ccout")
            nc.sync.dma_start(cc_in[:], pool_sb[:])
            nc.gpsimd.collective_compute(
                "AllReduce", ADD, replica_groups=[list(range(C))],
                ins=[cc_in[:].opt()], outs=[cc_out[:].opt()])
            pool_red = hp.tile([G, H], F32, tag="poolred")
            nc.sync.dma_start(pool_red[:], cc_out[:])
            pooled = hp.tile([G, H], BF16, tag="pooled")
            nc.vector.tensor_scalar(pooled[:], pool_red[:], invc_t[:], None,
                                    op0=MULT)

            ones_t = hp.tile([1, G], BF16, tag="ones")
            nc.vector.memset(ones_t[:], 1.0)
            bg_t = hp.tile([1, H], BF16, tag="bg")
            nc.sync.dma_start(bg_t[:], bg_d[:])
            pg = ps3.tile([G, H], F32, tag="ppool")
            KC = GF // 128
            for kc in range(KC):
                ga_c = sp.tile([128, G], BF16, tag="gac")
                nc.sync.dma_start(ga_c[:], gaT_d[kc * 128:(kc + 1) * 128, :])
                wg_c = sp.tile([128, H], BF16, tag="wgc")
                nc.sync.dma_start(wg_c[:], wg_d[kc * 128:(kc + 1) * 128, :])
                nc.tensor.matmul(pg[:], ga_c[:], wg_c[:],
                                 start=(kc == 0), stop=False)
            nc.tensor.matmul(pg[:], ones_t[:], bg_t[:], start=False, stop=True)
            g_sb = hp.tile([G, H], BF16, tag="gsb")
            nc.vector.tensor_copy(g_sb[:], pg[:])

            def transpose_to_sb(src_sb, tag):
                pt = ps2.tile([128, G], BF16, tag="aux")
                nc.tensor.transpose(pt[:], src_sb[:], ident[:G, :G])
                t_sb = hp.tile([128, G], BF16, tag=tag)
                nc.vector.tensor_copy(t_sb[:], pt[:])
                return t_sb
            pooledT = transpose_to_sb(pooled, "pooledT")
            gT = transpose_to_sb(g_sb, "gT")
            wc_t1 = hp.tile([H, H], BF16, tag="wc1")
            nc.sync.dma_start(wc_t1[:], wc_d[:128, :])
            wc_t2 = hp.tile([H, H], BF16, tag="wc2")
            nc.sync.dma_start(wc_t2[:], wc_d[128:, :])
            bc_t = hp.tile([1, H], BF16, tag="bc")
            nc.sync.dma_start(bc_t[:], bc_d[:])
            pc = ps3.tile([G, H], F32, tag="ppool")
            nc.tensor.matmul(pc[:], pooledT[:], wc_t1[:], start=True, stop=False)
            nc.tensor.matmul(pc[:], gT[:], wc_t2[:], start=False, stop=False)
            nc.tensor.matmul(pc[:], ones_t[:], bc_t[:], start=False, stop=True)
            comb = hp.tile([G, H], BF16, tag="comb")
            nc.scalar.activation(comb[:], pc[:], RELU)

            combT = transpose_to_sb(comb, "combT")
            wl_t = hp.tile([H, 1], BF16, tag="wl")
            nc.sync.dma_start(wl_t[:], wl_d[:])
            bl_t = hp.tile([1, 1], BF16, tag="bl")
            nc.sync.dma_start(bl_t[:], bl_d[:])
            po = ps3.tile([G, 1], F32, tag="ppool")
            nc.tensor.matmul(po[:], combT[:], wl_t[:], start=True, stop=False)
            nc.tensor.matmul(po[:], ones_t[:], bl_t[:], start=False, stop=True)
            o_sb = hp.tile([G, 1], F32, tag="osb")
            nc.vector.tensor_copy(o_sb[:], po[:])
            nc.sync.dma_start(out_d[:], o_sb[:])

    nc.compile()
    return nc


def _make_runner(nc, n_cores):
    import jax
    import concourse.mybir as mybir
    from concourse.bass2jax import (_bass_exec_p, partition_id_tensor,
                                    install_neuronx_cc_hook)
    from jax.sharding import Mesh, PartitionSpec
    from jax.experimental.shard_map import shard_map
    install_neuronx_cc_hook()
    partition_name = nc.partition_id_tensor.name if nc.partition_id_tensor else None
    in_names, out_names, out_avals, zero_outs = [], [], [], []
    for alloc in nc.m.functions[0].allocations:
        if not isinstance(alloc, mybir.MemoryLocationSet):
            continue
        name = alloc.memorylocations[0].name
        if alloc.kind == "ExternalInput":
            if name != partition_name:
                in_names.append(name)
        elif alloc.kind == "ExternalOutput":
            out_names.append(name)
            shape = tuple(alloc.tensor_shape)
            dtype = mybir.dt.np(alloc.dtype)
            out_avals.append(jax.core.ShapedArray(shape, dtype))
            zero_outs.append(np.zeros(shape, dtype))
    n_params, n_outs = len(in_names), len(out_avals)
    all_in = in_names + out_names + ([partition_name] if partition_name else [])

    def _body(*args):
        operands = list(args)
        if partition_name is not None:
            operands.append(partition_id_tensor())
        return tuple(_bass_exec_p.bind(
            *operands, out_avals=tuple(out_avals), in_names=tuple(all_in),
            out_names=tuple(out_names), lowering_input_output_aliases=(),
            sim_require_finite=True, sim_require_nnan=True, nc=nc))

    devices = jax.devices()[:n_cores]
    mesh = Mesh(np.asarray(devices), ("core",))
    sharded = jax.jit(
        shard_map(_body, mesh=mesh,
                  in_specs=(PartitionSpec("core"),) * (n_params + n_outs),
                  out_specs=(PartitionSpec("core"),) * n_outs,
                  check_rep=False),
        keep_unused=True)

    def run(in_maps):
        per_core = [[np.asarray(m[k]) for k in in_names] for m in in_maps]
        concat_in = [np.concatenate([per_core[c][i] for c in range(n_cores)], 0)
                     for i in range(n_params)]
        concat_zeros = [np.zeros((n_cores * z.shape[0], *z.shape[1:]), z.dtype)
                        for z in zero_outs]
        outs = sharded(*concat_in, *concat_zeros)
        jax.block_until_ready(outs)
        return [{name: np.asarray(outs[i]).reshape(n_cores, *out_avals[i].shape)[c]
                 for i, name in enumerate(out_names)} for c in range(n_cores)]

    return run, out_names


_CACHE = {}


def kernel(**inputs):
    meta, per_core, shared = _host_prep(inputs)
    key = (meta['KTOT'],)
    if key not in _CACHE:
        nc = _build(meta)
        run, _ = _make_runner(nc, C)
        _CACHE[key] = run
    run = _CACHE[key]
    in_maps = [{**shared, **pc} for pc in per_core]
    results = run(in_maps)
    return results[0]['out']


if __name__ == '__main__':
    pass


# revision 15
# speedup vs baseline: 1.6576x; 1.6576x over previous
"""GNN message-passing kernel for 8 Trainium2 NeuronCores.

Strategy: shard nodes/edges by destination-node range across 8 cores;
replicate weights; per layer all-gather the W_msg-transformed node table
(bf16) and gather per-edge rows via dma_gather; scatter-add via one-hot
matmuls on the tensor engine with PSUM accumulation per 128-node window.
One-hot scatter matrices are precomputed on the host and streamed from
DRAM (the DVE per-partition-scalar EQ build is ~1.6us/chunk on HW).
Edge-attr aggregate (computed once in layer 0), degree-bias and self
terms are folded into the same PSUM accumulation; ReLU evicts PSUM via
the scalar engine.
"""
import sys, os
for p in ('/opt/trn_rl_repo', '/root/.axon_site', '/root/.axon_site/_ro/trn_rl_repo',
          '/root/.axon_site/_ro/pypackages'):
    if os.path.isdir(p) and p not in sys.path:
        sys.path.append(p)

import numpy as np

# ---------------- problem constants (hardcoded) ----------------
N, E, G = 100000, 1000000, 64
F_IN, H, E_IN, GF = 84, 128, 6, 10368
C = 8                 # cores
SH = N // C           # 12500 real nodes per core
T = 98                # node chunks per core (ceil(12500/128))
SHP = T * 128         # 12544 padded nodes per core
NP = C * SHP          # padded global node space (100352)
NB = 4                # src buckets of 25088 rows (2 shards each, int16-safe)
BROWS = NP // NB      # 25088
W = T                 # dst windows per core (128 nodes each)
CALL = 2048           # slots per dma_gather call


def _host_prep(inputs):
    x = np.asarray(inputs['x'], np.float32)
    ei = np.asarray(inputs['edge_index']).astype(np.int64)
    ea = np.asarray(inputs['edge_attr'], np.float32)
    batch = np.asarray(inputs['batch']).astype(np.int64)

    src, dst = ei[0], ei[1]
    # table row: node (cs, r) -> cs*SHP + (r%128)*T + (r//128)  (partition-major)
    cs = src // SH
    r = src - cs * SH
    table_row = cs * SHP + (r % 128) * T + (r // 128)
    bucket = table_row // BROWS
    idx_in_bucket = (table_row - bucket * BROWS).astype(np.int16)

    core = dst // SH
    dst_local = dst - core * SH
    win = dst_local >> 7
    rel = dst_local & 127

    # per-(core,bucket,window) counts -> chunk plan (same on all cores)
    flat = (core * NB + bucket) * W + win
    cnt = np.bincount(flat, minlength=C * NB * W).reshape(C, NB, W)
    cell_chunks = -(-cnt.max(axis=0) // 128)          # [NB, W] (can be 0)
    cell_chunks = np.maximum(cell_chunks, 1)
    # bucket-major chunk layout for gather slots: (b, w) cells in w order
    cell_base = np.zeros((NB, W), np.int64)           # chunk base within bucket
    for b in range(NB):
        cell_base[b] = np.concatenate([[0], np.cumsum(cell_chunks[b])[:-1]])
    bucket_chunks = cell_chunks.sum(axis=1)           # [NB]
    bucket_base = np.concatenate([[0], np.cumsum(bucket_chunks)[:-1]])
    KTOT = int(bucket_chunks.sum())
    TOTS = KTOT * 128
    # window-major order for st/ea streams: chunks ordered (w, b, i)
    # global chunk id (bucket-major) -> position in window-major stream
    wm_of_bm = np.zeros(KTOT, np.int64)
    pos = 0
    win_nch = np.zeros(W, np.int64)
    for w in range(W):
        for b in range(NB):
            nch = int(cell_chunks[b, w])
            base = int(bucket_base[b] + cell_base[b, w])
            for i in range(nch):
                wm_of_bm[base + i] = pos
                pos += 1
            win_nch[w] += nch
    win_base = np.concatenate([[0], np.cumsum(win_nch)[:-1]])

    meta = dict(cell_chunks=cell_chunks, cell_base=cell_base,
                bucket_chunks=bucket_chunks, bucket_base=bucket_base,
                KTOT=KTOT, TOTS=TOTS, win_nch=win_nch, win_base=win_base)

    deg_real = np.bincount(dst, minlength=N)

    per_core = []
    order_all = np.lexsort((dst_local, bucket, core))
    core_sorted = core[order_all]
    core_starts = np.searchsorted(core_sorted, np.arange(C + 1))
    for c in range(C):
        sel = order_all[core_starts[c]:core_starts[c + 1]]
        b_c, w_c, rel_c = bucket[sel], win[sel], rel[sel]
        # rank within (b,w) cell (sel already sorted by (b, w, dst_local))
        cellid = b_c * W + w_c
        uniq, first_pos, counts = np.unique(cellid, return_index=True,
                                            return_counts=True)
        rank = np.arange(len(sel)) - np.repeat(first_pos, counts)
        slot = (bucket_base[b_c] + cell_base[b_c, w_c]) * 128 + rank
        chunk_bm = slot >> 7
        chunk_wm = wm_of_bm[chunk_bm]
        p_in_chunk = slot & 127

        idx_flat = np.zeros(TOTS, np.int16)
        idx_flat[slot] = idx_in_bucket[sel]
        idx16 = np.tile(idx_flat.reshape(-1, 16).T, (8, 1)).copy()  # [128,TOTS/16]

        # one-hot st: [128 p, KTOT(wm), 128 rel] bf16
        import ml_dtypes
        st = np.zeros((128, KTOT, 128), ml_dtypes.bfloat16)
        st[p_in_chunk, chunk_wm, rel_c] = 1.0
        st_flat = st.reshape(128, KTOT * 128)

        ea_arr = np.zeros((128, KTOT, 6), ml_dtypes.bfloat16)
        ea_arr[p_in_chunk, chunk_wm, :] = ea[sel].astype(ml_dtypes.bfloat16)
        ea_flat = ea_arr.reshape(128, KTOT * 6)

        # deg row [1, SHP], batch one-hot [128, T*G]
        degv = np.zeros((1, SHP), ml_dtypes.bfloat16)
        degv[0, :SH] = deg_real[c * SH:(c + 1) * SH].astype(ml_dtypes.bfloat16)
        ph = np.zeros((128, T, G), ml_dtypes.bfloat16)
        nn = np.arange(SH)
        ph[nn % 128, nn // 128, batch[c * SH:(c + 1) * SH]] = 1.0
        ph_flat = ph.reshape(128, T * G)

        xt = np.zeros((F_IN + 1, SHP), ml_dtypes.bfloat16)
        xt[:F_IN, :SH] = x[c * SH:(c + 1) * SH].T.astype(ml_dtypes.bfloat16)
        xt[F_IN, :] = 1.0
        per_core.append(dict(idx16=idx16, st=st_flat, ea=ea_flat,
                             deg=degv, ph=ph_flat, xt=xt))

    import ml_dtypes
    bf = lambda a: np.asarray(a, np.float32).astype(ml_dtypes.bfloat16)
    cntg = np.bincount(batch, minlength=G).astype(np.float32)
    shared = dict(invcnt=(1.0 / np.maximum(cntg, 1.0)).reshape(G, 1))

    def wedge_ext(we, bsum):
        return np.concatenate([np.asarray(we, np.float32),
                               np.asarray(bsum, np.float32).reshape(1, -1)], 0)
    shared['wmsg1'] = bf(inputs['W_msg1'])
    shared['wmsg2'] = bf(inputs['W_msg2'])
    shared['wmsg3'] = bf(inputs['W_msg3'])
    shared['wedge1'] = bf(wedge_ext(inputs['W_edge1'],
                                    np.asarray(inputs['b_edge1']) +
                                    np.asarray(inputs['b_msg1'])))
    shared['wedge2'] = bf(wedge_ext(inputs['W_edge2'],
                                    np.asarray(inputs['b_edge2']) +
                                    np.asarray(inputs['b_msg2'])))
    shared['wedge3'] = bf(wedge_ext(inputs['W_edge3'],
                                    np.asarray(inputs['b_edge3']) +
                                    np.asarray(inputs['b_msg3'])))
    shared['wself1'] = bf(np.concatenate(
        [np.asarray(inputs['W_self1'], np.float32),
         np.asarray(inputs['b_self1'], np.float32).reshape(1, -1)], 0))
    shared['gaT'] = bf(np.ascontiguousarray(
        np.asarray(inputs['graph_attr'], np.float32).T))          # [GF, 64]
    shared['wg'] = bf(inputs['W_g'])                               # [GF, H]
    shared['bg'] = bf(np.asarray(inputs['b_g']).reshape(1, H))
    shared['wc'] = bf(inputs['W_c'])                               # [256, H]
    shared['bc'] = bf(np.asarray(inputs['b_c']).reshape(1, H))
    shared['wl'] = bf(inputs['W_l'])                               # [H, 1]
    shared['bl'] = bf(np.asarray(inputs['b_l']).reshape(1, 1))
    return meta, per_core, shared


def _build(meta):
    import concourse.bacc as bacc
    import concourse.mybir as mybir
    import concourse.tile as tile
    from concourse.masks import make_identity
    F32, BF16, I16 = mybir.dt.float32, mybir.dt.bfloat16, mybir.dt.int16
    ADD = mybir.AluOpType.add
    MULT = mybir.AluOpType.mult
    RELU = mybir.ActivationFunctionType.Relu

    cell_chunks = meta['cell_chunks']
    cell_base = meta['cell_base']
    bucket_chunks = meta['bucket_chunks']
    bucket_base = meta['bucket_base']
    KTOT, TOTS = meta['KTOT'], meta['TOTS']
    win_nch, win_base = meta['win_nch'], meta['win_base']

    nc = bacc.Bacc("TRN2", target_bir_lowering=False, debug=False, num_devices=C,
                   num_swdge_queues=4)

    def inp(name, shape, dt=BF16):
        return nc.dram_tensor(name, shape, dt, kind="ExternalInput")
    xt_d = inp('xt', [F_IN + 1, SHP])
    idx_d = inp('idx16', [128, TOTS // 16], I16)
    st_d = inp('st', [128, KTOT * 128])
    ea_d = inp('ea', [128, KTOT * 6])
    deg_d = inp('deg', [1, SHP])
    ph_d = inp('ph', [128, T * G])
    invc_d = inp('invcnt', [G, 1], F32)
    wmsg_d = [inp('wmsg1', [F_IN, H]), inp('wmsg2', [H, H]), inp('wmsg3', [H, H])]
    wedge_d = [inp('wedge1', [7, H]), inp('wedge2', [7, H]), inp('wedge3', [7, H])]
    wself_d = inp('wself1', [F_IN + 1, H])
    gaT_d = inp('gaT', [GF, G])
    wg_d = inp('wg', [GF, H])
    bg_d = inp('bg', [1, H])
    wc_d = inp('wc', [2 * H, H])
    bc_d = inp('bc', [1, H])
    wl_d = inp('wl', [H, 1])
    bl_d = inp('bl', [1, 1])
    out_d = nc.dram_tensor('out', [G, 1], F32, kind="ExternalOutput")

    # gather call plan: round-robin across buckets by call index
    GCALLS = []
    for b in range(NB):
        sz = int(bucket_chunks[b]) * 128
        o = 0
        while o < sz:
            n_ = min(CALL, sz - o)
            GCALLS.append((b, o, n_))
            o += n_
    GCALLS.sort(key=lambda t: (t[1] // CALL, t[0]))

    with tile.TileContext(nc) as tc:
        NCHMAX = int(win_nch.max())
        with tc.tile_pool(name="cst", bufs=1) as cst, \
             tc.tile_pool(name="xbuf", bufs=1) as xbuf, \
             tc.tile_pool(name="xmp", bufs=1) as xmp, \
             tc.tile_pool(name="gp", bufs=2) as gp, \
             tc.tile_pool(name="stp", bufs=4) as stp, \
             tc.tile_pool(name="sp", bufs=4) as sp, \
             tc.tile_pool(name="hp", bufs=1) as hp, \
             tc.tile_pool(name="ps", bufs=3, space="PSUM") as ps, \
             tc.tile_pool(name="ps2", bufs=2, space="PSUM") as ps2, \
             tc.tile_pool(name="ps3", bufs=1, space="PSUM") as ps3, \
             tc.tile_pool(name="dram", bufs=1, space="DRAM") as dram:

            # ---- persistent SBUF data ----
            idx_t = cst.tile([128, TOTS // 16], I16)
            nc.sync.dma_start(idx_t[:], idx_d[:])
            ph_t = cst.tile([128, T, G], BF16)
            nc.sync.dma_start(ph_t[:], ph_d[:].rearrange("p (t g) -> p t g", g=G))
            invc_t = cst.tile([G, 1], F32)
            nc.sync.dma_start(invc_t[:], invc_d[:])
            ident = cst.tile([128, 128], BF16)
            make_identity(nc, ident[:])
            wmsg_t = []
            for l in range(3):
                kk = F_IN if l == 0 else H
                wt = cst.tile([kk, H], BF16, tag=f"wmsg{l}")
                nc.sync.dma_start(wt[:], wmsg_d[l][:])
                wmsg_t.append(wt)
            wedge_t = []
            for l in range(3):
                wt = cst.tile([7, H], BF16, tag=f"wedge{l}")
                nc.sync.dma_start(wt[:], wedge_d[l][:])
                wedge_t.append(wt)
            wedgeD0 = cst.tile([1, H], BF16, tag="wedgeD0")
            nc.sync.dma_start(wedgeD0[:], wedge_d[0][6:7, :])
            wself_t = cst.tile([F_IN + 1, H], BF16)
            nc.sync.dma_start(wself_t[:], wself_d[:])

            A = xbuf.tile([128, SHP], BF16, tag="A")
            B = xbuf.tile([128, SHP], BF16, tag="B")
            nc.vector.memset(A[:], 0.0)
            nc.sync.dma_start(A[:F_IN + 1, :], xt_d[:])

            xm_shard = dram.tile([SHP, H], BF16, tag="xmsh")
            xm_full = dram.tile([NP, H], BF16, tag="xmfull")
            agg7 = dram.tile([7, SHP], BF16, tag="agg7")   # rows 0-5 sum(ea), 6 deg

            def layer(l, XS, XD, resid, pool_ctx=None):
                KX = F_IN if l == 0 else H
                # 1) xm = XS @ Wmsg (bf16) -> DRAM shard -> AllGather
                xm_sb = xmp.tile([128, T, H], BF16, tag="xmsb")
                for t in range(T):
                    pxm = ps2.tile([128, H], F32, tag="aux")
                    nc.tensor.matmul(pxm[:], XS[:KX, t * 128:(t + 1) * 128],
                                     wmsg_t[l][:], start=True, stop=True)
                    nc.scalar.activation(xm_sb[:, t, :], pxm[:],
                                         mybir.ActivationFunctionType.Copy)
                nc.sync.dma_start(
                    xm_shard[:].rearrange("(p t) f -> p t f", p=128), xm_sb[:])
                nc.gpsimd.collective_compute(
                    "AllGather", mybir.AluOpType.bypass,
                    replica_groups=[list(range(C))],
                    ins=[xm_shard[:].opt()], outs=[xm_full[:].opt()])

                # 2) per-edge gathers (bucket streams, round-robin calls)
                gt = {}
                for (b, o, n_) in GCALLS:
                    g = gp.tile([128, CALL // 128, H], BF16, tag=f"g{b}")
                    base_slot = int(bucket_base[b]) * 128
                    nc.gpsimd.dma_gather(
                        g[:, :n_ // 128, :],
                        xm_full[b * BROWS:(b + 1) * BROWS, :],
                        idx_t[:, (base_slot + o) // 16:(base_slot + o + n_) // 16],
                        n_, n_, H, single_packet=False, queue_num=b)
                    for cch in range(n_ // 128):
                        gt[(b, (o // 128) + cch)] = g[:, cch, :]

                # 3) scatter windows with folded edge/deg/self terms
                for w in range(W):
                    nch = int(win_nch[w])
                    wb = int(win_base[w])
                    # stream st (and ea at l0) for this window's chunks
                    stw = stp.tile([128, NCHMAX, 128], BF16, tag="stw")
                    nc.sync.dma_start(
                        stw[:, :nch, :],
                        st_d[:, wb * 128:(wb + nch) * 128].rearrange(
                            "p (c f) -> p c f", f=128))
                    if l == 0:
                        eaw = stp.tile([128, NCHMAX, 6], BF16, tag="eaw")
                        nc.sync.dma_start(
                            eaw[:, :nch, :],
                            ea_d[:, wb * 6:(wb + nch) * 6].rearrange(
                                "p (c f) -> p c f", f=6))
                        degw = sp.tile([1, 128], BF16, tag="degw")
                        nc.sync.dma_start(degw[:], deg_d[:, w * 128:(w + 1) * 128])
                    else:
                        agw = sp.tile([7, 128], BF16, tag="agw")
                        nc.sync.dma_start(agw[:], agg7[:, w * 128:(w + 1) * 128])

                    px = ps.tile([128, 128], F32, tag="px")
                    if l == 0:
                        pe = ps2.tile([6, 128], F32, tag="aux")
                    k = 0
                    for b in range(NB):
                        cb = int(cell_base[b, w])
                        for i in range(int(cell_chunks[b, w])):
                            msg = gt[(b, cb + i)]
                            stc = stw[:, k, :]  # window-major k-th chunk
                            nc.tensor.matmul(px[:], msg, stc,
                                             start=(k == 0), stop=False)
                            if l == 0:
                                nc.tensor.matmul(pe[:], eaw[:, k, :], stc,
                                                 start=(k == 0), stop=(k == nch - 1))
                            k += 1
                    # fold terms
                    if l == 0:
                        pesb = sp.tile([6, 128], BF16, tag="pesb")
                        nc.vector.tensor_copy(pesb[:], pe[:])
                        nc.sync.dma_start(agg7[0:6, w * 128:(w + 1) * 128], pesb[:])
                        nc.sync.dma_start(agg7[6:7, w * 128:(w + 1) * 128], degw[:])
                        nc.tensor.matmul(px[:], wedge_t[0][0:6, :], pesb[:],
                                         start=False, stop=False)
                        nc.tensor.matmul(px[:], wedgeD0[:], degw[:],
                                         start=False, stop=False)
                        nc.tensor.matmul(px[:], wself_t[:],
                                         A[:F_IN + 1, w * 128:(w + 1) * 128],
                                         start=False, stop=True)
                    else:
                        nc.tensor.matmul(px[:], wedge_t[l][:], agw[:],
                                         start=False, stop=False)
                        nc.tensor.matmul(px[:], ident[:],
                                         XS[:, w * 128:(w + 1) * 128],
                                         start=False, stop=True)
                    wc_ = slice(w * 128, (w + 1) * 128)
                    nc.scalar.activation(XD[:, wc_], px[:], RELU)
                    if resid:
                        nc.vector.tensor_tensor(out=XD[:, wc_], in0=XD[:, wc_],
                                                in1=XS[:, wc_], op=ADD)
                    if pool_ctx is not None:
                        ppool_, ph_ = pool_ctx
                        ptr = ps2.tile([128, 128], BF16, tag="aux")
                        nc.tensor.transpose(ptr[:], XD[:, wc_], ident[:])
                        xts = sp.tile([128, 128], BF16, tag="xts")
                        nc.scalar.activation(xts[:], ptr[:],
                                             mybir.ActivationFunctionType.Copy)
                        nc.tensor.matmul(ppool_[:], ph_[:, w, :], xts[:],
                                         start=(w == 0), stop=(w == W - 1))

            layer(0, A, B, False)   # x1 in B
            layer(1, B, A, True)    # x2 in A
            ppool = ps3.tile([G, H], F32, tag="ppool")
            layer(2, A, B, True, pool_ctx=(ppool, ph_t))   # x3 in B, pooled

            # -------- pooling reduce + head --------
            pool_sb = hp.tile([G, H], F32, tag="poolsb")
            nc.vector.tensor_copy(pool_sb[:], ppool[:])
            cc_in = dram.tile([G, H], F32, tag="ccin")
            cc_out = dram.tile([G, H], F32, tag="ccout")
            nc.sync.dma_start(cc_in[:], pool_sb[:])
            nc.gpsimd.collective_compute(
                "AllReduce", ADD, replica_groups=[list(range(C))],
                ins=[cc_in[:].opt()], outs=[cc_out[:].opt()])
            pool_red = hp.tile([G, H], F32, tag="poolred")
            nc.sync.dma_start(pool_red[:], cc_out[:])
            pooled = hp.tile([G, H], BF16, tag="pooled")
            nc.vector.tensor_scalar(pooled[:], pool_red[:], invc_t[:], None,
                                    op0=MULT)

            ones_t = hp.tile([1, G], BF16, tag="ones")
            nc.vector.memset(ones_t[:], 1.0)
            bg_t = hp.tile([1, H], BF16, tag="bg")
            nc.sync.dma_start(bg_t[:], bg_d[:])
            pg = ps3.tile([G, H], F32, tag="ppool")
            KC = GF // 128
            for kc in range(KC):
                ga_c = sp.tile([128, G], BF16, tag="gac")
                nc.sync.dma_start(ga_c[:], gaT_d[kc * 128:(kc + 1) * 128, :])
                wg_c = sp.tile([128, H], BF16, tag="wgc")
                nc.sync.dma_start(wg_c[:], wg_d[kc * 128:(kc + 1) * 128, :])
                nc.tensor.matmul(pg[:], ga_c[:], wg_c[:],
                                 start=(kc == 0), stop=False)
            nc.tensor.matmul(pg[:], ones_t[:], bg_t[:], start=False, stop=True)
            g_sb = hp.tile([G, H], BF16, tag="gsb")
            nc.vector.tensor_copy(g_sb[:], pg[:])

            def transpose_to_sb(src_sb, tag):
                pt = ps2.tile([128, G], BF16, tag="aux")
                nc.tensor.transpose(pt[:], src_sb[:], ident[:G, :G])
                t_sb = hp.tile([128, G], BF16, tag=tag)
                nc.vector.tensor_copy(t_sb[:], pt[:])
                return t_sb
            pooledT = transpose_to_sb(pooled, "pooledT")
            gT = transpose_to_sb(g_sb, "gT")
            wc_t1 = hp.tile([H, H], BF16, tag="wc1")
            nc.sync.dma_start(wc_t1[:], wc_d[:128, :])
            wc_t2 = hp.tile([H, H], BF16, tag="wc2")
            nc.sync.dma_start(wc_t2[:], wc_d[128:, :])
            bc_t = hp.tile([1, H], BF16, tag="bc")
            nc.sync.dma_start(bc_t[:], bc_d[:])
            pc = ps3.tile([G, H], F32, tag="ppool")
            nc.tensor.matmul(pc[:], pooledT[:], wc_t1[:], start=True, stop=False)
            nc.tensor.matmul(pc[:], gT[:], wc_t2[:], start=False, stop=False)
            nc.tensor.matmul(pc[:], ones_t[:], bc_t[:], start=False, stop=True)
            comb = hp.tile([G, H], BF16, tag="comb")
            nc.scalar.activation(comb[:], pc[:], RELU)

            combT = transpose_to_sb(comb, "combT")
            wl_t = hp.tile([H, 1], BF16, tag="wl")
            nc.sync.dma_start(wl_t[:], wl_d[:])
            bl_t = hp.tile([1, 1], BF16, tag="bl")
            nc.sync.dma_start(bl_t[:], bl_d[:])
            po = ps3.tile([G, 1], F32, tag="ppool")
            nc.tensor.matmul(po[:], combT[:], wl_t[:], start=True, stop=False)
            nc.tensor.matmul(po[:], ones_t[:], bl_t[:], start=False, stop=True)
            o_sb = hp.tile([G, 1], F32, tag="osb")
            nc.vector.tensor_copy(o_sb[:], po[:])
            nc.sync.dma_start(out_d[:], o_sb[:])

    nc.compile()
    return nc


def _make_runner(nc, n_cores):
    import jax
    import concourse.mybir as mybir
    from concourse.bass2jax import (_bass_exec_p, partition_id_tensor,
                                    install_neuronx_cc_hook)
    from jax.sharding import Mesh, PartitionSpec
    from jax.experimental.shard_map import shard_map
    install_neuronx_cc_hook()
    partition_name = nc.partition_id_tensor.name if nc.partition_id_tensor else None
    in_names, out_names, out_avals, zero_outs = [], [], [], []
    for alloc in nc.m.functions[0].allocations:
        if not isinstance(alloc, mybir.MemoryLocationSet):
            continue
        name = alloc.memorylocations[0].name
        if alloc.kind == "ExternalInput":
            if name != partition_name:
                in_names.append(name)
        elif alloc.kind == "ExternalOutput":
            out_names.append(name)
            shape = tuple(alloc.tensor_shape)
            dtype = mybir.dt.np(alloc.dtype)
            out_avals.append(jax.core.ShapedArray(shape, dtype))
            zero_outs.append(np.zeros(shape, dtype))
    n_params, n_outs = len(in_names), len(out_avals)
    all_in = in_names + out_names + ([partition_name] if partition_name else [])

    def _body(*args):
        operands = list(args)
        if partition_name is not None:
            operands.append(partition_id_tensor())
        return tuple(_bass_exec_p.bind(
            *operands, out_avals=tuple(out_avals), in_names=tuple(all_in),
            out_names=tuple(out_names), lowering_input_output_aliases=(),
            sim_require_finite=True, sim_require_nnan=True, nc=nc))

    devices = jax.devices()[:n_cores]
    mesh = Mesh(np.asarray(devices), ("core",))
    sharded = jax.jit(
        shard_map(_body, mesh=mesh,
                  in_specs=(PartitionSpec("core"),) * (n_params + n_outs),
                  out_specs=(PartitionSpec("core"),) * n_outs,
                  check_rep=False),
        keep_unused=True)

    def run(in_maps):
        per_core = [[np.asarray(m[k]) for k in in_names] for m in in_maps]
        concat_in = [np.concatenate([per_core[c][i] for c in range(n_cores)], 0)
                     for i in range(n_params)]
        concat_zeros = [np.zeros((n_cores * z.shape[0], *z.shape[1:]), z.dtype)
                        for z in zero_outs]
        outs = sharded(*concat_in, *concat_zeros)
        jax.block_until_ready(outs)
        return [{name: np.asarray(outs[i]).reshape(n_cores, *out_avals[i].shape)[c]
                 for i, name in enumerate(out_names)} for c in range(n_cores)]

    return run, out_names


_CACHE = {}


def kernel(**inputs):
    meta, per_core, shared = _host_prep(inputs)
    key = (meta['KTOT'],)
    if key not in _CACHE:
        nc = _build(meta)
        run, _ = _make_runner(nc, C)
        _CACHE[key] = run
    run = _CACHE[key]
    in_maps = [{**shared, **pc} for pc in per_core]
    results = run(in_maps)
    return results[0]['out']


if __name__ == '__main__':
    pass


# revision 17
# speedup vs baseline: 1.7573x; 1.0601x over previous
"""GNN message-passing kernel for 8 Trainium2 NeuronCores.

Strategy: shard nodes/edges by destination-node range across 8 cores;
replicate weights; per layer all-gather the W_msg-transformed node table
(bf16) and gather per-edge rows via dma_gather; scatter-add via one-hot
matmuls on the tensor engine with PSUM accumulation per 128-node window.
One-hot scatter matrices are precomputed on the host and streamed from
DRAM (the DVE per-partition-scalar EQ build is ~1.6us/chunk on HW).
Edge-attr aggregate (computed once in layer 0), degree-bias and self
terms are folded into the same PSUM accumulation; ReLU evicts PSUM via
the scalar engine.
"""
import sys, os
for p in ('/opt/trn_rl_repo', '/root/.axon_site', '/root/.axon_site/_ro/trn_rl_repo',
          '/root/.axon_site/_ro/pypackages'):
    if os.path.isdir(p) and p not in sys.path:
        sys.path.append(p)

import numpy as np

# ---------------- problem constants (hardcoded) ----------------
N, E, G = 100000, 1000000, 64
F_IN, H, E_IN, GF = 84, 128, 6, 10368
C = 8                 # cores
SH = N // C           # 12500 real nodes per core
T = 98                # node chunks per core (ceil(12500/128))
SHP = T * 128         # 12544 padded nodes per core
NP = C * SHP          # padded global node space (100352)
NB = 4                # src buckets of 25088 rows (2 shards each, int16-safe)
BROWS = NP // NB      # 25088
W = T                 # dst windows per core (128 nodes each)
CALL = 2048           # slots per dma_gather call


def _host_prep(inputs):
    x = np.asarray(inputs['x'], np.float32)
    ei = np.asarray(inputs['edge_index']).astype(np.int64)
    ea = np.asarray(inputs['edge_attr'], np.float32)
    batch = np.asarray(inputs['batch']).astype(np.int64)

    src, dst = ei[0], ei[1]
    # table row: node (cs, r) -> cs*SHP + (r%128)*T + (r//128)  (partition-major)
    cs = src // SH
    r = src - cs * SH
    table_row = cs * SHP + (r % 128) * T + (r // 128)
    bucket = table_row // BROWS
    idx_in_bucket = (table_row - bucket * BROWS).astype(np.int16)

    core = dst // SH
    dst_local = dst - core * SH
    win = dst_local >> 7
    rel = dst_local & 127

    # per-(core,bucket,window) counts -> chunk plan (same on all cores)
    flat = (core * NB + bucket) * W + win
    cnt = np.bincount(flat, minlength=C * NB * W).reshape(C, NB, W)
    cell_chunks = -(-cnt.max(axis=0) // 128)          # [NB, W] (can be 0)
    cell_chunks = np.maximum(cell_chunks, 1)
    # bucket-major chunk layout for gather slots: (b, w) cells in w order
    cell_base = np.zeros((NB, W), np.int64)           # chunk base within bucket
    for b in range(NB):
        cell_base[b] = np.concatenate([[0], np.cumsum(cell_chunks[b])[:-1]])
    bucket_chunks = cell_chunks.sum(axis=1)           # [NB]
    bucket_base = np.concatenate([[0], np.cumsum(bucket_chunks)[:-1]])
    KTOT = int(bucket_chunks.sum())
    TOTS = KTOT * 128
    # window-major order for st/ea streams: chunks ordered (w, b, i)
    # global chunk id (bucket-major) -> position in window-major stream
    wm_of_bm = np.zeros(KTOT, np.int64)
    pos = 0
    win_nch = np.zeros(W, np.int64)
    for w in range(W):
        for b in range(NB):
            nch = int(cell_chunks[b, w])
            base = int(bucket_base[b] + cell_base[b, w])
            for i in range(nch):
                wm_of_bm[base + i] = pos
                pos += 1
            win_nch[w] += nch
    win_base = np.concatenate([[0], np.cumsum(win_nch)[:-1]])

    meta = dict(cell_chunks=cell_chunks, cell_base=cell_base,
                bucket_chunks=bucket_chunks, bucket_base=bucket_base,
                KTOT=KTOT, TOTS=TOTS, win_nch=win_nch, win_base=win_base)

    deg_real = np.bincount(dst, minlength=N)

    per_core = []
    order_all = np.lexsort((dst_local, bucket, core))
    core_sorted = core[order_all]
    core_starts = np.searchsorted(core_sorted, np.arange(C + 1))
    for c in range(C):
        sel = order_all[core_starts[c]:core_starts[c + 1]]
        b_c, w_c, rel_c = bucket[sel], win[sel], rel[sel]
        # rank within (b,w) cell (sel already sorted by (b, w, dst_local))
        cellid = b_c * W + w_c
        uniq, first_pos, counts = np.unique(cellid, return_index=True,
                                            return_counts=True)
        rank = np.arange(len(sel)) - np.repeat(first_pos, counts)
        slot = (bucket_base[b_c] + cell_base[b_c, w_c]) * 128 + rank
        chunk_bm = slot >> 7
        chunk_wm = wm_of_bm[chunk_bm]
        p_in_chunk = slot & 127

        idx_flat = np.zeros(TOTS, np.int16)
        idx_flat[slot] = idx_in_bucket[sel]
        idx16 = np.tile(idx_flat.reshape(-1, 16).T, (8, 1)).copy()  # [128,TOTS/16]

        # one-hot st: [128 p, KTOT(wm), 128 rel] bf16
        import ml_dtypes
        st = np.zeros((128, KTOT, 128), ml_dtypes.bfloat16)
        st[p_in_chunk, chunk_wm, rel_c] = 1.0
        st_flat = st.reshape(128, KTOT * 128)

        # static per-dst aggregates: rows 0-5 sum(edge_attr), row 6 degree
        dl = dst_local[sel]
        agg7v = np.zeros((7, SHP), np.float32)
        for f in range(E_IN):
            agg7v[f, :SH] = np.bincount(dl, weights=ea[sel][:, f], minlength=SH)
        agg7v[6, :SH] = deg_real[c * SH:(c + 1) * SH]
        agg7v = agg7v.astype(ml_dtypes.bfloat16)
        ph = np.zeros((128, T, G), ml_dtypes.bfloat16)
        nn = np.arange(SH)
        ph[nn % 128, nn // 128, batch[c * SH:(c + 1) * SH]] = 1.0
        ph_flat = ph.reshape(128, T * G)

        xt = np.zeros((F_IN + 1, SHP), ml_dtypes.bfloat16)
        xt[:F_IN, :SH] = x[c * SH:(c + 1) * SH].T.astype(ml_dtypes.bfloat16)
        xt[F_IN, :] = 1.0
        per_core.append(dict(idx16=idx16, st=st_flat, agg7=agg7v,
                             ph=ph_flat, xt=xt))

    import ml_dtypes
    bf = lambda a: np.asarray(a, np.float32).astype(ml_dtypes.bfloat16)
    cntg = np.bincount(batch, minlength=G).astype(np.float32)
    shared = dict(invcnt=(1.0 / np.maximum(cntg, 1.0)).reshape(G, 1))

    def wedge_ext(we, bsum):
        return np.concatenate([np.asarray(we, np.float32),
                               np.asarray(bsum, np.float32).reshape(1, -1)], 0)
    shared['wmsg1'] = bf(inputs['W_msg1'])
    shared['wmsg2'] = bf(inputs['W_msg2'])
    shared['wmsg3'] = bf(inputs['W_msg3'])
    shared['wedge1'] = bf(wedge_ext(inputs['W_edge1'],
                                    np.asarray(inputs['b_edge1']) +
                                    np.asarray(inputs['b_msg1'])))
    shared['wedge2'] = bf(wedge_ext(inputs['W_edge2'],
                                    np.asarray(inputs['b_edge2']) +
                                    np.asarray(inputs['b_msg2'])))
    shared['wedge3'] = bf(wedge_ext(inputs['W_edge3'],
                                    np.asarray(inputs['b_edge3']) +
                                    np.asarray(inputs['b_msg3'])))
    shared['wself1'] = bf(np.concatenate(
        [np.asarray(inputs['W_self1'], np.float32),
         np.asarray(inputs['b_self1'], np.float32).reshape(1, -1)], 0))
    shared['gaT'] = bf(np.ascontiguousarray(
        np.asarray(inputs['graph_attr'], np.float32).T))          # [GF, 64]
    shared['wg'] = bf(inputs['W_g'])                               # [GF, H]
    shared['bg'] = bf(np.asarray(inputs['b_g']).reshape(1, H))
    shared['wc'] = bf(inputs['W_c'])                               # [256, H]
    shared['bc'] = bf(np.asarray(inputs['b_c']).reshape(1, H))
    shared['wl'] = bf(inputs['W_l'])                               # [H, 1]
    shared['bl'] = bf(np.asarray(inputs['b_l']).reshape(1, 1))
    return meta, per_core, shared


def _build(meta):
    import concourse.bacc as bacc
    import concourse.mybir as mybir
    import concourse.tile as tile
    from concourse.masks import make_identity
    F32, BF16, I16 = mybir.dt.float32, mybir.dt.bfloat16, mybir.dt.int16
    ADD = mybir.AluOpType.add
    MULT = mybir.AluOpType.mult
    RELU = mybir.ActivationFunctionType.Relu

    cell_chunks = meta['cell_chunks']
    cell_base = meta['cell_base']
    bucket_chunks = meta['bucket_chunks']
    bucket_base = meta['bucket_base']
    KTOT, TOTS = meta['KTOT'], meta['TOTS']
    win_nch, win_base = meta['win_nch'], meta['win_base']

    nc = bacc.Bacc("TRN2", target_bir_lowering=False, debug=False, num_devices=C,
                   num_swdge_queues=4)

    def inp(name, shape, dt=BF16):
        return nc.dram_tensor(name, shape, dt, kind="ExternalInput")
    xt_d = inp('xt', [F_IN + 1, SHP])
    idx_d = inp('idx16', [128, TOTS // 16], I16)
    st_d = inp('st', [128, KTOT * 128])
    agg7_d = inp('agg7', [7, SHP])
    ph_d = inp('ph', [128, T * G])
    invc_d = inp('invcnt', [G, 1], F32)
    wmsg_d = [inp('wmsg1', [F_IN, H]), inp('wmsg2', [H, H]), inp('wmsg3', [H, H])]
    wedge_d = [inp('wedge1', [7, H]), inp('wedge2', [7, H]), inp('wedge3', [7, H])]
    wself_d = inp('wself1', [F_IN + 1, H])
    gaT_d = inp('gaT', [GF, G])
    wg_d = inp('wg', [GF, H])
    bg_d = inp('bg', [1, H])
    wc_d = inp('wc', [2 * H, H])
    bc_d = inp('bc', [1, H])
    wl_d = inp('wl', [H, 1])
    bl_d = inp('bl', [1, 1])
    out_d = nc.dram_tensor('out', [G, 1], F32, kind="ExternalOutput")

    # gather call plan: round-robin across buckets by call index
    GCALLS = []
    for b in range(NB):
        sz = int(bucket_chunks[b]) * 128
        o = 0
        while o < sz:
            n_ = min(CALL, sz - o)
            GCALLS.append((b, o, n_))
            o += n_
    GCALLS.sort(key=lambda t: (t[1] // CALL, t[0]))

    with tile.TileContext(nc) as tc:
        NCHMAX = int(win_nch.max())
        GNMAX = int(max(win_nch[w:min(w + 4, W)].sum() for w in range(0, W, 4)))
        with tc.tile_pool(name="cst", bufs=1) as cst, \
             tc.tile_pool(name="xbuf", bufs=1) as xbuf, \
             tc.tile_pool(name="xmp", bufs=1) as xmp, \
             tc.tile_pool(name="gp", bufs=2) as gp, \
             tc.tile_pool(name="stp", bufs=3) as stp, \
             tc.tile_pool(name="sp", bufs=4) as sp, \
             tc.tile_pool(name="hp", bufs=1) as hp, \
             tc.tile_pool(name="ps", bufs=3, space="PSUM") as ps, \
             tc.tile_pool(name="ps2", bufs=2, space="PSUM") as ps2, \
             tc.tile_pool(name="ps3", bufs=1, space="PSUM") as ps3, \
             tc.tile_pool(name="dram", bufs=1, space="DRAM") as dram:

            # ---- persistent SBUF data ----
            idx_t = cst.tile([128, TOTS // 16], I16)
            nc.sync.dma_start(idx_t[:], idx_d[:])
            ph_t = cst.tile([128, T, G], BF16)
            nc.sync.dma_start(ph_t[:], ph_d[:].rearrange("p (t g) -> p t g", g=G))
            invc_t = cst.tile([G, 1], F32)
            nc.sync.dma_start(invc_t[:], invc_d[:])
            ident = cst.tile([128, 128], BF16)
            make_identity(nc, ident[:])
            wmsg_t = []
            for l in range(3):
                kk = F_IN if l == 0 else H
                wt = cst.tile([kk, H], BF16, tag=f"wmsg{l}")
                nc.sync.dma_start(wt[:], wmsg_d[l][:])
                wmsg_t.append(wt)
            wedge_t = []
            for l in range(3):
                wt = cst.tile([7, H], BF16, tag=f"wedge{l}")
                nc.sync.dma_start(wt[:], wedge_d[l][:])
                wedge_t.append(wt)
            wedgeD0 = cst.tile([1, H], BF16, tag="wedgeD0")
            nc.sync.dma_start(wedgeD0[:], wedge_d[0][6:7, :])
            wself_t = cst.tile([F_IN + 1, H], BF16)
            nc.sync.dma_start(wself_t[:], wself_d[:])

            agg7_sb = cst.tile([7, SHP], BF16)
            nc.sync.dma_start(agg7_sb[:], agg7_d[:])
            A = xbuf.tile([128, SHP], BF16, tag="A")
            B = xbuf.tile([128, SHP], BF16, tag="B")
            nc.vector.memset(A[:], 0.0)
            nc.sync.dma_start(A[:F_IN + 1, :], xt_d[:])

            xm_shard = dram.tile([SHP, H], BF16, tag="xmsh")
            xm_full = dram.tile([NP, H], BF16, tag="xmfull")

            def layer(l, XS, XD, resid, pool_ctx=None):
                KX = F_IN if l == 0 else H
                # 1) xm = XS @ Wmsg (bf16) -> DRAM shard -> AllGather
                xm_sb = xmp.tile([128, T, H], BF16, tag="xmsb")
                for t in range(T):
                    pxm = ps2.tile([128, H], F32, tag="aux")
                    nc.tensor.matmul(pxm[:], XS[:KX, t * 128:(t + 1) * 128],
                                     wmsg_t[l][:], start=True, stop=True)
                    nc.scalar.activation(xm_sb[:, t, :], pxm[:],
                                         mybir.ActivationFunctionType.Copy)
                nc.sync.dma_start(
                    xm_shard[:].rearrange("(p t) f -> p t f", p=128), xm_sb[:])
                nc.gpsimd.collective_compute(
                    "AllGather", mybir.AluOpType.bypass,
                    replica_groups=[list(range(C))],
                    ins=[xm_shard[:].opt()], outs=[xm_full[:].opt()])

                # 2) per-edge gathers (bucket streams, round-robin calls)
                gt = {}
                for (b, o, n_) in GCALLS:
                    g = gp.tile([128, CALL // 128, H], BF16, tag=f"g{b}")
                    base_slot = int(bucket_base[b]) * 128
                    nc.gpsimd.dma_gather(
                        g[:, :n_ // 128, :],
                        xm_full[b * BROWS:(b + 1) * BROWS, :],
                        idx_t[:, (base_slot + o) // 16:(base_slot + o + n_) // 16],
                        n_, n_, H, single_packet=False, queue_num=b)
                    for cch in range(n_ // 128):
                        gt[(b, (o // 128) + cch)] = g[:, cch, :]

                # 3) scatter windows with folded edge/deg/self terms;
                # st streamed in 4-window groups
                WGRP = 4
                for w in range(W):
                    nch = int(win_nch[w])
                    wb = int(win_base[w])
                    if w % WGRP == 0:
                        g0 = int(win_base[w])
                        wlast = min(w + WGRP, W) - 1
                        gn = int(win_base[wlast] + win_nch[wlast]) - g0
                        stg = stp.tile([128, GNMAX, 128], BF16, tag="stg")
                        nc.sync.dma_start(
                            stg[:, :gn, :],
                            st_d[:, g0 * 128:(g0 + gn) * 128].rearrange(
                                "p (c f) -> p c f", f=128))
                    px = ps.tile([128, 128], F32, tag="px")
                    k = 0
                    for b in range(NB):
                        cb = int(cell_base[b, w])
                        for i in range(int(cell_chunks[b, w])):
                            msg = gt[(b, cb + i)]
                            stc = stg[:, wb - g0 + k, :]
                            nc.tensor.matmul(px[:], msg, stc,
                                             start=(k == 0), stop=False)
                            k += 1
                    # fold terms
                    nc.tensor.matmul(px[:], wedge_t[l][:],
                                     agg7_sb[:, w * 128:(w + 1) * 128],
                                     start=False, stop=False)
                    if l == 0:
                        nc.tensor.matmul(px[:], wself_t[:],
                                         A[:F_IN + 1, w * 128:(w + 1) * 128],
                                         start=False, stop=True)
                    else:
                        nc.tensor.matmul(px[:], ident[:],
                                         XS[:, w * 128:(w + 1) * 128],
                                         start=False, stop=True)
                    wc_ = slice(w * 128, (w + 1) * 128)
                    nc.scalar.activation(XD[:, wc_], px[:], RELU)
                    if resid:
                        nc.vector.tensor_tensor(out=XD[:, wc_], in0=XD[:, wc_],
                                                in1=XS[:, wc_], op=ADD)
                    if pool_ctx is not None:
                        ppool_, ph_ = pool_ctx
                        ptr = ps2.tile([128, 128], BF16, tag="aux")
                        nc.tensor.transpose(ptr[:], XD[:, wc_], ident[:])
                        xts = sp.tile([128, 128], BF16, tag="xts")
                        nc.scalar.activation(xts[:], ptr[:],
                                             mybir.ActivationFunctionType.Copy)
                        nc.tensor.matmul(ppool_[:], ph_[:, w, :], xts[:],
                                         start=(w == 0), stop=(w == W - 1))

            layer(0, A, B, False)   # x1 in B
            layer(1, B, A, True)    # x2 in A
            ppool = ps3.tile([G, H], F32, tag="ppool")
            layer(2, A, B, True, pool_ctx=(ppool, ph_t))   # x3 in B, pooled

            # -------- pooling reduce + head --------
            pool_sb = hp.tile([G, H], F32, tag="poolsb")
            nc.vector.tensor_copy(pool_sb[:], ppool[:])
            cc_in = dram.tile([G, H], F32, tag="ccin")
            cc_out = dram.tile([G, H], F32, tag="ccout")
            nc.sync.dma_start(cc_in[:], pool_sb[:])
            nc.gpsimd.collective_compute(
                "AllReduce", ADD, replica_groups=[list(range(C))],
                ins=[cc_in[:].opt()], outs=[cc_out[:].opt()])
            pool_red = hp.tile([G, H], F32, tag="poolred")
            nc.sync.dma_start(pool_red[:], cc_out[:])
            pooled = hp.tile([G, H], BF16, tag="pooled")
            nc.vector.tensor_scalar(pooled[:], pool_red[:], invc_t[:], None,
                                    op0=MULT)

            ones_t = hp.tile([1, G], BF16, tag="ones")
            nc.vector.memset(ones_t[:], 1.0)
            bg_t = hp.tile([1, H], BF16, tag="bg")
            nc.sync.dma_start(bg_t[:], bg_d[:])
            pg = ps3.tile([G, H], F32, tag="ppool")
            KC = GF // 128
            for kc in range(KC):
                ga_c = sp.tile([128, G], BF16, tag="gac")
                nc.sync.dma_start(ga_c[:], gaT_d[kc * 128:(kc + 1) * 128, :])
                wg_c = sp.tile([128, H], BF16, tag="wgc")
                nc.sync.dma_start(wg_c[:], wg_d[kc * 128:(kc + 1) * 128, :])
                nc.tensor.matmul(pg[:], ga_c[:], wg_c[:],
                                 start=(kc == 0), stop=False)
            nc.tensor.matmul(pg[:], ones_t[:], bg_t[:], start=False, stop=True)
            g_sb = hp.tile([G, H], BF16, tag="gsb")
            nc.vector.tensor_copy(g_sb[:], pg[:])

            def transpose_to_sb(src_sb, tag):
                pt = ps2.tile([128, G], BF16, tag="aux")
                nc.tensor.transpose(pt[:], src_sb[:], ident[:G, :G])
                t_sb = hp.tile([128, G], BF16, tag=tag)
                nc.vector.tensor_copy(t_sb[:], pt[:])
                return t_sb
            pooledT = transpose_to_sb(pooled, "pooledT")
            gT = transpose_to_sb(g_sb, "gT")
            wc_t1 = hp.tile([H, H], BF16, tag="wc1")
            nc.sync.dma_start(wc_t1[:], wc_d[:128, :])
            wc_t2 = hp.tile([H, H], BF16, tag="wc2")
            nc.sync.dma_start(wc_t2[:], wc_d[128:, :])
            bc_t = hp.tile([1, H], BF16, tag="bc")
            nc.sync.dma_start(bc_t[:], bc_d[:])
            pc = ps3.tile([G, H], F32, tag="ppool")
            nc.tensor.matmul(pc[:], pooledT[:], wc_t1[:], start=True, stop=False)
            nc.tensor.matmul(pc[:], gT[:], wc_t2[:], start=False, stop=False)
            nc.tensor.matmul(pc[:], ones_t[:], bc_t[:], start=False, stop=True)
            comb = hp.tile([G, H], BF16, tag="comb")
            nc.scalar.activation(comb[:], pc[:], RELU)

            combT = transpose_to_sb(comb, "combT")
            wl_t = hp.tile([H, 1], BF16, tag="wl")
            nc.sync.dma_start(wl_t[:], wl_d[:])
            bl_t = hp.tile([1, 1], BF16, tag="bl")
            nc.sync.dma_start(bl_t[:], bl_d[:])
            po = ps3.tile([G, 1], F32, tag="ppool")
            nc.tensor.matmul(po[:], combT[:], wl_t[:], start=True, stop=False)
            nc.tensor.matmul(po[:], ones_t[:], bl_t[:], start=False, stop=True)
            o_sb = hp.tile([G, 1], F32, tag="osb")
            nc.vector.tensor_copy(o_sb[:], po[:])
            nc.sync.dma_start(out_d[:], o_sb[:])

    nc.compile()
    return nc


def _make_runner(nc, n_cores):
    import jax
    import concourse.mybir as mybir
    from concourse.bass2jax import (_bass_exec_p, partition_id_tensor,
                                    install_neuronx_cc_hook)
    from jax.sharding import Mesh, PartitionSpec
    from jax.experimental.shard_map import shard_map
    install_neuronx_cc_hook()
    partition_name = nc.partition_id_tensor.name if nc.partition_id_tensor else None
    in_names, out_names, out_avals, zero_outs = [], [], [], []
    for alloc in nc.m.functions[0].allocations:
        if not isinstance(alloc, mybir.MemoryLocationSet):
            continue
        name = alloc.memorylocations[0].name
        if alloc.kind == "ExternalInput":
            if name != partition_name:
                in_names.append(name)
        elif alloc.kind == "ExternalOutput":
            out_names.append(name)
            shape = tuple(alloc.tensor_shape)
            dtype = mybir.dt.np(alloc.dtype)
            out_avals.append(jax.core.ShapedArray(shape, dtype))
            zero_outs.append(np.zeros(shape, dtype))
    n_params, n_outs = len(in_names), len(out_avals)
    all_in = in_names + out_names + ([partition_name] if partition_name else [])

    def _body(*args):
        operands = list(args)
        if partition_name is not None:
            operands.append(partition_id_tensor())
        return tuple(_bass_exec_p.bind(
            *operands, out_avals=tuple(out_avals), in_names=tuple(all_in),
            out_names=tuple(out_names), lowering_input_output_aliases=(),
            sim_require_finite=True, sim_require_nnan=True, nc=nc))

    devices = jax.devices()[:n_cores]
    mesh = Mesh(np.asarray(devices), ("core",))
    sharded = jax.jit(
        shard_map(_body, mesh=mesh,
                  in_specs=(PartitionSpec("core"),) * (n_params + n_outs),
                  out_specs=(PartitionSpec("core"),) * n_outs,
                  check_rep=False),
        keep_unused=True)

    def run(in_maps):
        per_core = [[np.asarray(m[k]) for k in in_names] for m in in_maps]
        concat_in = [np.concatenate([per_core[c][i] for c in range(n_cores)], 0)
                     for i in range(n_params)]
        concat_zeros = [np.zeros((n_cores * z.shape[0], *z.shape[1:]), z.dtype)
                        for z in zero_outs]
        outs = sharded(*concat_in, *concat_zeros)
        jax.block_until_ready(outs)
        return [{name: np.asarray(outs[i]).reshape(n_cores, *out_avals[i].shape)[c]
                 for i, name in enumerate(out_names)} for c in range(n_cores)]

    return run, out_names


_CACHE = {}


def kernel(**inputs):
    meta, per_core, shared = _host_prep(inputs)
    key = (meta['KTOT'],)
    if key not in _CACHE:
        nc = _build(meta)
        run, _ = _make_runner(nc, C)
        _CACHE[key] = run
    run = _CACHE[key]
    in_maps = [{**shared, **pc} for pc in per_core]
    results = run(in_maps)
    return results[0]['out']


if __name__ == '__main__':
    pass
